# revision 44
# baseline (speedup 1.0000x reference)
"""Trainium2 Bass kernel for CrossAttention (B=4, N=M=2048, H=8, D=64,
Q_DIM=1024, C_DIM=768).

Sharding over 8 cores: core c handles batch b = c//2 and head-group
hg = c%2 (4 heads, 256 inner dims).  Each core computes a *partial*
output projection (its 256 inner dims of the 512 total); the host sums
core pairs and adds the output bias.

Device-side layouts are all matmul-native (out = lhsT.T @ rhs):
  - qT/kT [inner, seq]: computed with weight-chunk stationary, xT/ctxT
    moving.  v [keys, inner] with ctxT-chunk stationary, WvT moving.
  - scores are computed transposed: S.T[keys, q] = kT_h contracted with
    qT_h over the head dim, so softmax's key-reduction is a matmul
    reduction and no on-chip transposes are needed anywhere.
  - V carries an appended ones-column, so the P.T@V matmul also yields the
    per-query softmax denominators (row 64 of the [65, q] accumulator).
  - denominators are broadcast across partitions with a K=1 ones matmul,
    reciprocal'd on VectorE, and folded into the PSUM->SBUF copy of the
    attention output.
  - all matmul inputs are float32r (single-pass FP22 multiply, fp32
    accumulate) for 4x tensor-engine throughput vs true fp32.

Schedule (this revision): the kernel is a static schedule tuned against
the concourse timeline cost model.
  - DMA order is chosen so the first attention head's inputs (wk-m0,
    ctx keys 0-1023, wv, wq-m0, x0) land first; the context is split
    into key-halves so h0's first 8 key-chunks can start while the
    second half streams.
  - kT(m0,K1), v(kc0-7) and qT(m0,qb0) are computed in the prologue
    DMA shadow.
  - the remaining projection work (kT m0-K2 / m1, qT m1, qT qb1, the
    final projections of the previous query block, v kc8-15) is spread
    across the attention inner loops in per-(h,kc) filler slots, sized
    so the PE tracks the Activation engine's exp chain.
  - the last head of the last block normalizes in two query-halves so
    the final projections can start early; output evictions alternate
    Act/DVE so the tail pipeline isn't serialized on one engine.

The attention mask in this problem is all-True; if a mask with False
entries is ever passed, kernel() falls back to a numpy reference.
"""

import numpy as np

B, N, M = 4, 2048, 2048
Q_DIM, C_DIM, H, D = 1024, 768, 8, 64
INNER = H * D  # 512
SCALE = D ** -0.5

N_CORES = 8
H_PER_CORE = 4          # heads per core
IN_PER_CORE = H_PER_CORE * D  # 256 inner dims per core
QB = 1024               # query block
N_QB = N // QB          # 2
KC = M // 128           # 16 key chunks
QK_CHUNKS = Q_DIM // 128   # 8
CK_CHUNKS = C_DIM // 128   # 6
IN_CHUNKS = IN_PER_CORE // 128  # 2

_CACHED_NC = None


def _build_bass():
    import concourse.bass as bass
    import concourse.mybir as mybir
    import concourse.tile as tile
    from concourse import bacc

    f32r = mybir.dt.float32r
    f32 = mybir.dt.float32
    ts, ds = bass.ts, bass.ds
    Exp = mybir.ActivationFunctionType.Exp

    nc = bacc.Bacc("TRN2", target_bir_lowering=False)

    xT = nc.dram_tensor("xT", [Q_DIM, N], f32r, kind="ExternalInput")
    cT = nc.dram_tensor("cT", [C_DIM, M], f32r, kind="ExternalInput")
    wq = nc.dram_tensor("wq", [Q_DIM, IN_PER_CORE], f32r, kind="ExternalInput")
    wk = nc.dram_tensor("wk", [C_DIM, IN_PER_CORE], f32r, kind="ExternalInput")
    wv = nc.dram_tensor("wv", [C_DIM, IN_PER_CORE], f32r, kind="ExternalInput")
    wo = nc.dram_tensor("wo", [IN_PER_CORE, Q_DIM], f32r, kind="ExternalInput")
    out_d = nc.dram_tensor("out", [N, Q_DIM], f32, kind="ExternalOutput")

    with tile.TileContext(nc) as tc:
        with (
            tc.tile_pool(name="persist", bufs=1) as persist,
            tc.tile_pool(name="stream", bufs=2) as stream,
            tc.tile_pool(name="psA", bufs=2, space="PSUM") as psA,
            tc.tile_pool(name="psB", bufs=2, space="PSUM") as psB,
        ):
            wk_r = wk.rearrange("(k p) n -> p k n", p=128)
            wq_r = wq.rearrange("(k p) n -> p k n", p=128)

            # ---- constants ----
            onesf = persist.tile([128, D], f32, tag="onesf")
            nc.vector.memset(onesf, 1.0)
            ones_sb = persist.tile([D + 1, D], f32r, tag="ones")
            nc.vector.tensor_copy(out=ones_sb, in_=onesf[0 : D + 1, :])

            # ---- DMA schedule (execution order == emission order) ----
            wk_sb = persist.tile([128, CK_CHUNKS, IN_PER_CORE], f32r, tag="wk")
            nc.sync.dma_start(out=wk_sb, in_=wk_r)

            ctx_sb = persist.tile([128, CK_CHUNKS, M], f32r, tag="ctx")
            for c in range(CK_CHUNKS):
                nc.sync.dma_start(
                    out=ctx_sb[:, c, 0:1024], in_=cT[ds(c * 128, 128), 0:1024]
                )

            wv_sb = persist.tile([128, CK_CHUNKS, IN_PER_CORE], f32r, tag="wv")
            nc.sync.dma_start(out=wv_sb, in_=wv.rearrange("(k p) n -> p k n", p=128))

            wq_sb = persist.tile([128, QK_CHUNKS, IN_PER_CORE], f32r, tag="wq")
            nc.sync.dma_start(out=wq_sb[:, :, 0:128], in_=wq_r[:, :, 0:128])

            x0_sb = stream.tile([128, QK_CHUNKS, QB], f32r, tag="x", bufs=1,
                                name="x0")
            for k in range(QK_CHUNKS):
                nc.sync.dma_start(
                    out=x0_sb[:, k, :], in_=xT[ds(k * 128, 128), 0:QB]
                )

            nc.sync.dma_start(out=wq_sb[:, :, 128:256], in_=wq_r[:, :, 128:256])
            for c in range(CK_CHUNKS):
                nc.sync.dma_start(
                    out=ctx_sb[:, c, 1024:2048], in_=cT[ds(c * 128, 128), 1024:2048]
                )

            wo_sb = persist.tile([128, IN_CHUNKS, Q_DIM], f32r, tag="wo")
            nc.sync.dma_start(out=wo_sb, in_=wo.rearrange("(t p) n -> p t n", p=128))

            # x1 shares the x slot; its DMA waits on the slot-release sem at
            # runtime (qT qb1 reads pace with per-chunk arrival), so it is
            # last in the DMA program order.
            x1_sb = stream.tile([128, QK_CHUNKS, QB], f32r, tag="x", bufs=1,
                                name="x1")
            for k in range(QK_CHUNKS):
                nc.sync.dma_start(
                    out=x1_sb[:, k, :], in_=xT[ds(k * 128, 128), QB : 2 * QB]
                )

            # ---- persistent compute targets ----
            kT_sb = persist.tile([128, IN_CHUNKS, M], f32r, tag="kt")
            v_sb = persist.tile([128, KC, H_PER_CORE, D + 1], f32r, tag="v")
            nc.vector.tensor_copy(
                out=v_sb[:, :, :, D : D + 1],
                in_=onesf.rearrange("p (a b c) -> p a b c", a=KC, b=H_PER_CORE),
            )

            # -- kT(m, half) j-quarter: one [128,512] psum tile, 6 matmuls --
            # whole-width variant for the prologue (psA 4KB slots)
            def make_kt(m, half, pool, tag, evict_act=False):
                st8 = {}

                def part(clo, chi):
                    if "ps" not in st8:
                        st8["ps"] = pool.tile([128, 1024], f32, tag=tag,
                                              name=f"kps{m}{half}")
                    kps = st8["ps"]
                    for c in range(clo, chi):
                        for j in range(2):
                            nc.tensor.matmul(
                                kps[:, ts(j, 512)],
                                wk_sb[:, c, ts(m, 128)],
                                ctx_sb[:, c, ds(half * 1024 + j * 512, 512)],
                                start=(c == 0),
                                stop=(c == CK_CHUNKS - 1),
                            )

                def evict():
                    dst = kT_sb[:, m, ds(half * 1024, 1024)]
                    if evict_act:
                        nc.scalar.copy(out=dst, in_=st8["ps"])
                    else:
                        nc.vector.tensor_copy(out=dst, in_=st8["ps"])

                return part, evict

            # 512-key filler visit: computes kT_sb[:, m, q512*512 : +512]
            # in two <=3-matmul parts so no single slot gets a PE clump.
            def make_kt_q(m, q512, evict_act=False):
                st8 = {}

                def part(clo, chi, evict=False):
                    if "ps" not in st8:
                        st8["ps"] = psB.tile([128, 512], f32, tag="B", bufs=2,
                                             name=f"kq{m}{q512}")
                    kps = st8["ps"]
                    for c in range(clo, chi):
                        nc.tensor.matmul(
                            kps,
                            wk_sb[:, c, ts(m, 128)],
                            ctx_sb[:, c, ts(q512, 512)],
                            start=(c == 0),
                            stop=(c == CK_CHUNKS - 1),
                        )
                    if evict:
                        dst = kT_sb[:, m, ts(q512, 512)]
                        if evict_act:
                            nc.scalar.copy(out=dst, in_=kps)
                        else:
                            nc.vector.tensor_copy(out=dst, in_=kps)

                return (lambda: part(0, 3), lambda: part(3, 6, evict=True))

            # -- incremental qT(m) for a query block --
            def make_qt(qT_sb, x_sb, m, qb, pool, tag, evict_act=False):
                st8 = {}

                def part(klo, khi):
                    if "ps" not in st8:
                        st8["ps"] = pool.tile([128, QB], f32, tag=tag,
                                              name=f"qps{qb}{m}")
                    qps = st8["ps"]
                    for k in range(klo, khi):
                        for j in range(2):
                            nc.tensor.matmul(
                                qps[:, ts(j, 512)],
                                wq_sb[:, k, ts(m, 128)],
                                x_sb[:, k, ts(j, 512)],
                                start=(k == 0),
                                stop=(k == QK_CHUNKS - 1),
                            )

                def evict():
                    if evict_act:
                        nc.scalar.copy(out=qT_sb[:, m, :], in_=st8["ps"])
                    else:
                        nc.vector.tensor_copy(out=qT_sb[:, m, :], in_=st8["ps"])

                return part, evict

            def emit_v_chunk(kc, pool, tag, bufs=None):
                vps = pool.tile([128, IN_PER_CORE], f32, tag=tag, bufs=bufs,
                                name=f"vps{kc}")
                for c in range(CK_CHUNKS):
                    nc.tensor.matmul(
                        vps,
                        ctx_sb[:, c, ts(kc, 128)],
                        wv_sb[:, c, :],
                        start=(c == 0),
                        stop=(c == CK_CHUNKS - 1),
                    )
                nc.vector.tensor_copy(
                    out=v_sb[:, kc, :, 0:D],
                    in_=vps.rearrange("p (h d) -> p h d", h=H_PER_CORE),
                )

            # qT j-half filler: computes qT_sb[:, m, jh*512 : +512] in two
            # 4-matmul parts.
            def make_qt_j(qT_sb, x_sb, m, qb, jh):
                st8 = {}

                def part(klo, khi, evict=False):
                    if "ps" not in st8:
                        st8["ps"] = psB.tile([128, 512], f32, tag="B", bufs=2,
                                             name=f"qj{qb}{m}{jh}")
                    qps = st8["ps"]
                    for k in range(klo, khi):
                        nc.tensor.matmul(
                            qps,
                            wq_sb[:, k, ts(m, 128)],
                            x_sb[:, k, ts(jh, 512)],
                            start=(k == 0),
                            stop=(k == QK_CHUNKS - 1),
                        )
                    if evict:
                        nc.vector.tensor_copy(out=qT_sb[:, m, ts(jh, 512)],
                                              in_=qps)

                return (lambda: part(0, 4), lambda: part(4, 8, evict=True))

            # ---- prologue compute (runs in the DMA shadow) ----
            # kT(m0,K1) and kT(m1,K1) interleave per ctx-chunk arrival.
            kt00_part, kt00_evict = make_kt(0, 0, psA, "A")
            kt10_part, kt10_evict = make_kt(1, 0, psA, "A")
            for c in range(CK_CHUNKS):
                kt00_part(c, c + 1)
                kt10_part(c, c + 1)
            kt00_evict()
            kt10_evict()

            # v(kc0-7) and qT(m0,qb0) interleave: v paces with wv/ctx, qT
            # with the x0 chunks.
            qT0_sb = stream.tile([128, IN_CHUNKS, QB], f32r, tag="qt", bufs=1,
                                 name="qT0")
            qt00_part, qt00_evict = make_qt(qT0_sb, x0_sb, 0, 0, psA, "A",
                                            evict_act=True)
            for k in range(QK_CHUNKS):
                emit_v_chunk(k, psA, "A")
                qt00_part(k, k + 1)
            qt00_evict()

            qT1_sb = stream.tile([128, IN_CHUNKS, QB], f32r, tag="qt2", bufs=1,
                                 name="qT1")
            qT_tiles = [qT0_sb, qT1_sb]

            # ---- filler slot table ----
            # fill[(qb, h, kc)] -> list of closures emitted after exp(h, kc)
            fill = {}

            def add_fill(qb, h, kc, fn):
                fill.setdefault((qb, h, kc), []).append(fn)

            # Everything key-half-2 flavored is gated on the ctx-K2 DMAs,
            # which land ~30-37us (after x0); nothing K2-gated may be
            # emitted before h0-kc5 or it head-of-line blocks the PE queue.
            # kT(m0, K2) quarters: q2 gates S(h0, kc8) (Act reaches it
            # ~37.8us), q3 gates S(h0, kc12).  Evict on Act so it lands
            # right before exp(kc8)/exp(kc12) in the Act queue.
            # qT(m1, qb0) as h0's first fillers (kc0-3 carry nothing else);
            # gates only S(h2).  wq-m1 lands right after x0.
            qt01a, qt01b = make_qt_j(qT0_sb, x0_sb, 1, 0, 0)
            qt01c, qt01d = make_qt_j(qT0_sb, x0_sb, 1, 0, 1)
            add_fill(0, 0, 0, qt01a)
            add_fill(0, 0, 1, qt01b)
            add_fill(0, 0, 2, qt01c)
            add_fill(0, 0, 3, qt01d)

            ktq02a, ktq02b = make_kt_q(0, 2, evict_act=True)
            ktq03a, ktq03b = make_kt_q(0, 3, evict_act=True)
            add_fill(0, 0, 4, ktq02a)
            add_fill(0, 0, 5, ktq02b)
            add_fill(0, 0, 8, ktq03a)
            add_fill(0, 0, 9, ktq03b)

            # v kc8-15 singles (also K2-gated): PV(kc) is emitted at slot
            # kc+1 before that slot's fillers, so v(kc) sits at slot <= kc.
            for kc0, slot in ((8, 7), (9, 8), (10, 10), (11, 11), (12, 12),
                              (13, 13), (14, 14), (15, 15)):
                add_fill(0, 0, slot, lambda kc=kc0:
                         emit_v_chunk(kc, psB, "B", bufs=2))

            # qT(m1, qb0): gates S(h2).  wq-m1 lands ~39us.  Spread one
            # part per two slots so the PV stream never lags the pt ring.

            # kT(m1) 512-key quarters: q0/q1 gate S(h2, kc0/kc4); q2/q3
            # gate S(h2, kc8/kc12).  q3 spills into h2's early slots.
            # kT(m1, K2) quarters (K1 done in the prologue): gate
            # S(h2, kc8/kc12).
            ktq1 = [make_kt_q(1, q) for q in (2, 3)]
            add_fill(0, 1, 4, ktq1[0][0])
            add_fill(0, 1, 5, ktq1[0][1])
            add_fill(0, 1, 7, ktq1[1][0])
            add_fill(0, 1, 8, ktq1[1][1])

            # qT(qb1, m0): gates qb1-h0.  x1 chunks land as the x slot
            # frees (~48us + 1.5us/chunk -> all in by ~60us).
            qt10a, qt10b = make_qt_j(qT1_sb, x1_sb, 0, 1, 0)
            qt10c, qt10d = make_qt_j(qT1_sb, x1_sb, 0, 1, 1)
            add_fill(0, 2, 3, qt10a)
            add_fill(0, 2, 5, qt10b)
            add_fill(0, 2, 7, qt10c)
            add_fill(0, 2, 9, qt10d)

            # qT(qb1, m1): gates only qb1-h2, so it lives in qb1-h0/h1's
            # even slots (the odd ones carry qb0's final projections).
            qt11a, qt11b = make_qt_j(qT1_sb, x1_sb, 1, 1, 0)
            qt11c, qt11d = make_qt_j(qT1_sb, x1_sb, 1, 1, 1)
            add_fill(0, 3, 1, qt11a)
            add_fill(0, 3, 3, qt11b)
            add_fill(0, 3, 5, qt11c)
            add_fill(0, 3, 7, qt11d)

            # ---- final projection: one j-half of one 128-query chunk ----
            ost_tiles = {}

            def emit_final_half(qb, qm, jh, ot_all, evict_eng="dve", tag="B"):
                ops = psB.tile([128, 512], f32, tag=tag, bufs=2,
                               name=f"ops{qb}{qm}{jh}")
                for t in range(IN_CHUNKS):
                    nc.tensor.matmul(
                        ops,
                        ot_all[:, t, ts(qm, 128)],
                        wo_sb[:, t, ts(jh, 512)],
                        start=(t == 0),
                        stop=(t == IN_CHUNKS - 1),
                    )
                if (qb, qm) not in ost_tiles:
                    ost_tiles[(qb, qm)] = stream.tile(
                        [128, Q_DIM], f32, tag="ost", bufs=3, name=f"ost{qb}{qm}"
                    )
                ost = ost_tiles[(qb, qm)]
                if evict_eng == "act":
                    nc.scalar.copy(out=ost[:, ts(jh, 512)], in_=ops)
                else:
                    nc.vector.tensor_copy(out=ost[:, ts(jh, 512)], in_=ops)
                if jh == 1:
                    # SP (sync) HWDGE: SP is idle after the input loads
                    nc.sync.dma_start(
                        out=out_d[ds(qb * QB + qm * 128, 128), :], in_=ost
                    )

            # full-width final chunk on the psA ring -- for the tail, where
            # the score pipeline is done and psA is free.
            def emit_final_full(qb, qm, ot_all, evict_eng="dve"):
                ops = psA.tile([128, Q_DIM], f32, tag="A", name=f"opf{qb}{qm}")
                for t in range(IN_CHUNKS):
                    for j in range(2):
                        nc.tensor.matmul(
                            ops[:, ts(j, 512)],
                            ot_all[:, t, ts(qm, 128)],
                            wo_sb[:, t, ts(j, 512)],
                            start=(t == 0),
                            stop=(t == IN_CHUNKS - 1),
                        )
                ost = stream.tile([128, Q_DIM], f32, tag="ost", bufs=3,
                                  name=f"osf{qb}{qm}")
                if evict_eng == "act":
                    nc.scalar.copy(out=ost, in_=ops)
                else:
                    nc.vector.tensor_copy(out=ost, in_=ops)
                nc.sync.dma_start(
                    out=out_d[ds(qb * QB + qm * 128, 128), :], in_=ost
                )

            # ---- attention: one flat software-pipelined (qb, h, kc) stream --
            ot_alls = {}
            for qb in range(N_QB):
                ot_alls[qb] = stream.tile([128, IN_CHUNKS, QB], f32r,
                                          tag="otall", bufs=2, name=f"otall{qb}")
                # previous block's final chunk halves spread across this
                # block's h0-h2 loops.
                if qb > 0:
                    slots = ([(0, kc) for kc in range(1, 10, 2)]
                             + [(1, kc) for kc in range(1, 10, 2)]
                             + [(2, kc) for kc in range(1, 12, 2)])
                    for qm in range(QB // 128):
                        for jh in range(2):
                            h, kc = slots[qm * 2 + jh]
                            add_fill(qb, h, kc,
                                     lambda qb=qb, qm=qm, jh=jh:
                                     emit_final_half(qb - 1, qm, jh,
                                                     ot_alls[qb - 1]))

            def emit_pv(p):
                qb, h, kc, pt, ot_j = p
                for j in range(2):
                    nc.tensor.matmul(
                        ot_j[j],
                        v_sb[:, kc, h, :],
                        pt[:, ts(j, 512)],
                        start=(kc == 0),
                        stop=(kc == KC - 1),
                    )

            def emit_normalize(qb, h, ot_j, tail=False):
                t, po = h // 2, (h % 2) * D
                if tail:
                    # 256-wide pieces: shortest chain from last PV to the
                    # first final projection.
                    for s in range(4):
                        ot_raw = stream.tile([D + 1, 256], f32r, tag="otraw",
                                             bufs=2, name=f"otrz{qb}{h}{s}")
                        nc.scalar.copy(out=ot_raw, in_=ot_j[s // 2][:, ds((s % 2) * 256, 256)])
                        bc_ps = psB.tile([D, 256], f32, tag="B", bufs=2,
                                         name=f"bcz{qb}{h}{s}")
                        nc.tensor.matmul(bc_ps, ones_sb[D : D + 1, :],
                                         ot_raw[D : D + 1, :],
                                         start=True, stop=True)
                        nc.vector.reciprocal(out=bc_ps, in_=bc_ps)
                        nc.vector.tensor_mul(
                            out=ot_alls[qb][po : po + D, t, ds(s * 256, 256)],
                            in0=ot_raw[0:D, :],
                            in1=bc_ps,
                        )
                        for qm in range(s * 2, s * 2 + 2):
                            if qm % 2 == 0:
                                emit_final_full(qb, qm, ot_alls[qb],
                                                evict_eng="dve")
                            else:
                                hr = "B" if qm % 4 == 1 else "ot"
                                emit_final_half(qb, qm, 0, ot_alls[qb],
                                                evict_eng="act", tag=hr)
                                emit_final_half(qb, qm, 1, ot_alls[qb],
                                                evict_eng="act", tag=hr)
                    return
                for s in range(2):
                    ot_raw = stream.tile([D + 1, 512], f32r, tag="otraw",
                                         bufs=2, name=f"otraw{qb}{h}{s}")
                    if tail:
                        nc.scalar.copy(out=ot_raw, in_=ot_j[s])
                    else:
                        nc.vector.tensor_copy(out=ot_raw, in_=ot_j[s])
                    bc_ps = psB.tile([D, 512], f32, tag="B", bufs=2,
                                     name=f"bc{qb}{h}{s}")
                    nc.tensor.matmul(
                        bc_ps,
                        ones_sb[D : D + 1, :],
                        ot_raw[D : D + 1, :],
                        start=True,
                        stop=True,
                    )
                    nc.vector.reciprocal(out=bc_ps, in_=bc_ps)
                    nc.vector.tensor_mul(
                        out=ot_alls[qb][po : po + D, t, ds(s * 512, 512)],
                        in0=ot_raw[0:D, :],
                        in1=bc_ps,
                    )

            # Flat lead-2 stream: at step g the scores for step g+2 are
            # emitted first, so the Act queue always holds two ready exps
            # and fillers can never starve it; the st ring (2 bufs) then
            # throttles the PE to the exp chain's pace.
            steps = [(qb, h, kc)
                     for qb in range(N_QB)
                     for h in range(H_PER_CORE)
                     for kc in range(KC)]
            ot_js = {}  # (qb, h) -> [ot_j0, ot_j1]
            pts = {}    # step index -> pt tile

            def emit_S(g):
                qb, h, kc = steps[g]
                t, po = h // 2, (h % 2) * D
                st = psA.tile([128, QB], f32, tag="A", name=f"st{qb}{h}{kc}")
                for j in range(2):
                    nc.tensor.matmul(
                        st[:, ts(j, 512)],
                        kT_sb[po : po + D, t, ts(kc, 128)],
                        qT_tiles[qb][po : po + D, t, ts(j, 512)],
                        start=True,
                        stop=True,
                    )
                return st

            def emit_PV(g):
                qb, h, kc = steps[g]
                if (qb, h) not in ot_js:
                    ot_js[(qb, h)] = [
                        psB.tile([D + 1, 512], f32, tag="ot", bufs=2,
                                 name=f"ot{qb}{h}{j}")
                        for j in range(2)
                    ]
                emit_pv((qb, h, kc, pts.pop(g), ot_js[(qb, h)]))
                if kc == KC - 1:
                    emit_normalize(qb, h, ot_js.pop((qb, h)),
                                   tail=(g == len(steps) - 1))

            sts = {0: emit_S(0), 1: emit_S(1)}
            for g, (qb, h, kc) in enumerate(steps):
                if g + 2 < len(steps):
                    sts[g + 2] = emit_S(g + 2)
                if g >= 1:
                    emit_PV(g - 1)
                pt = stream.tile([128, QB], f32r, tag="pt", bufs=4,
                                 name=f"pt{qb}{h}{kc}")
                nc.scalar.activation(out=pt, in_=sts.pop(g), func=Exp,
                                     scale=SCALE)
                pts[g] = pt
                for fn in fill.pop((qb, h, kc), ()):
                    if fn is not None:
                        fn()

            # flush the very last step
            emit_PV(len(steps) - 1)

    nc.finalize()
    return nc


def _get_nc():
    global _CACHED_NC
    if _CACHED_NC is None:
        _CACHED_NC = _build_bass()
    return _CACHED_NC


def _numpy_fallback(x, context, mask, Wq, Wk, Wv, Wout, bout):
    q = (x @ Wq.T).reshape(B, N, H, D)
    k = (context @ Wk.T).reshape(B, M, H, D)
    v = (context @ Wv.T).reshape(B, M, H, D)
    sim = np.einsum("bnhd,bmhd->bhnm", q, k) * SCALE
    sim = np.where(mask[:, None, None, :], sim, -np.finfo(np.float32).max)
    sim -= sim.max(axis=-1, keepdims=True)
    attn = np.exp(sim)
    attn /= attn.sum(axis=-1, keepdims=True)
    out = np.einsum("bhnm,bmhd->bnhd", attn, v).reshape(B, N, INNER)
    return (out @ Wout.T + bout).astype(np.float32)


def kernel(x, context, mask, Wq, Wk, Wv, Wout, bout, _want_results=False):
    x = np.asarray(x, dtype=np.float32)
    context = np.asarray(context, dtype=np.float32)
    mask = np.asarray(mask)
    Wq = np.asarray(Wq, dtype=np.float32)
    Wk = np.asarray(Wk, dtype=np.float32)
    Wv = np.asarray(Wv, dtype=np.float32)
    Wout = np.asarray(Wout, dtype=np.float32)
    bout = np.asarray(bout, dtype=np.float32)

    if not mask.all():
        return _numpy_fallback(x, context, mask, Wq, Wk, Wv, Wout, bout)

    from concourse.bass_utils import run_bass_kernel_spmd

    in_maps = []
    for c in range(N_CORES):
        b, hg = c // 2, c % 2
        sl = slice(hg * IN_PER_CORE, (hg + 1) * IN_PER_CORE)
        in_maps.append(
            {
                "xT": np.ascontiguousarray(x[b].T),
                "cT": np.ascontiguousarray(context[b].T),
                "wq": np.ascontiguousarray(Wq[sl, :].T),
                "wk": np.ascontiguousarray(Wk[sl, :].T),
                "wv": np.ascontiguousarray(Wv[sl, :].T),
                "wo": np.ascontiguousarray(Wout[:, sl].T),
            }
        )

    res = run_bass_kernel_spmd(_get_nc(), in_maps, core_ids=list(range(N_CORES)))

    out = np.empty((B, N, Q_DIM), dtype=np.float32)
    for b in range(B):
        out[b] = res.results[2 * b]["out"] + res.results[2 * b + 1]["out"] + bout
    if _want_results:
        return out, res
    return out
